# revision 23
# baseline (speedup 1.0000x reference)
"""Trainium2 Bass kernel for the Chebyshev spectral layer.

Computation (per reference):
  x_cheb = DCT-I(x)[..., :512];  om = einsum('bix,iox->box', x_cheb, w)
  out = IDCT-I(pad(om))  ==  om @ M2  with M2[k,n] = cos(pi*k*n/2047)*c2[k]

Sharding: data-parallel over batch. 8 cores; the batch is processed in
CHUNKS pipelined waves of NCORES*BPC batches so that the upload of wave
k+1 overlaps the execute + download of wave k (the axon tunnel is
full-duplex, so h2d and d2h stream concurrently). The DCT matrices and
weights are replicated device-resident constants; all wire traffic is
fp16 (accumulation stays f32 in PSUM).

Per-core dataflow per wave (fp16 operands, f32 psum):
  T1  PE-transpose x [bi,n] -> XT [n,bi] in 128x128 blocks
  S1  x_cheb psum [bi,k] = sum_j XT_j.T @ M1T_j
      evacuate with partition-shifted copies -> XC2 [(k2,i), b, kc]
  S2  per-mode kc: psum[(k2,o), b] = Wbd[:,:,kc].T @ XC2[:,:,kc]
      (block-diag fp16 weights, 2 modes per matmul)
  T2  PE-transpose -> OMT [kl, ch, b%2, o]
  S3  out psum [(b2,o), n] = sum_ch OMT[:,ch,:,:].T @ M2P[:,ch,:]

Execution path: the Bass module is lowered through the bass_exec PJRT
custom call exactly once per process (AOT fast-dispatch compile of a
shard_map over 8 cores); the executable and all input-independent
device buffers are cached in module state, so steady-state kernel()
calls only pay fp16 conversion + wire transfer + dispatch.
"""
import numpy as np

import jax
from jax.sharding import Mesh, PartitionSpec, NamedSharding
from jax.experimental.shard_map import shard_map

import concourse.bass as bass
import concourse.tile as tile
from concourse import mybir
from concourse.bass2jax import (
    _bass_exec_p,
    fast_dispatch_compile,
    install_neuronx_cc_hook,
    partition_id_tensor,
)
from concourse.vector_clock import ScopedClock

F32 = mybir.dt.float32
F32R = mybir.dt.float32r
BF16 = mybir.dt.bfloat16
FP16 = mybir.dt.float16

B, IC, OC, NG, MD = 64, 64, 64, 2048, 512
NCORES = 8
BPC = 4                     # batches per core per wave
NBC = NCORES * BPC          # batches per wave (32)
CHUNKS = B // NBC           # pipelined waves (2)
P = 128

_CACHE = {}


class SplitDrainTC(tile.TileContext):
    """Walrus in this container rejects >1 sync-wait per instruction. Split
    extra waits onto same-engine NoOps emitted immediately before the
    instruction (identical semantics: conjunction of sem waits in program
    order)."""

    MAX_WAITS = 1

    def _add_instruction(self, inst):
        si = inst.sync_info
        if si is not None and si.on_wait and len(si.on_wait) > self.MAX_WAITS:
            waits = list(si.on_wait)
            si.on_wait = waits[: self.MAX_WAITS]
            for w in waits[self.MAX_WAITS:]:
                nop = mybir.InstNoOp(
                    name=self.nc.get_next_instruction_name(), ins=[], outs=[]
                )
                nop.engine = inst.engine
                nop.sync_info = mybir.SyncInfo(on_wait=[w], on_update=[])
                super()._add_instruction(nop)
        super()._add_instruction(inst)

    def _drain_and_barrier(self, tick_clock, wait_clock):
        drain_inst = self.nc.sync.drain()
        wait_clock.add_sem_waits(
            drain_inst.ins, ScopedClock({None: tick_clock.global_clock})
        )
        si = drain_inst.ins.sync_info
        waits = list(si.on_wait or []) if si else []
        if len(waits) > 1:
            si.on_wait = waits[:1]
            for w in waits[1:]:
                d2 = self.nc.sync.drain()
                d2.ins.sync_info = mybir.SyncInfo(on_wait=[w], on_update=[])
        self.nc.all_engine_barrier()
        popped = self.nc._tile_sem_poison_stack.pop()
        assert popped is self._sem_poison
        self.nc.clear_and_free_semaphores(list(self.sems.allocated().values()))
        self.nc.all_engine_barrier()


def _constants():
    if "m1t" in _CACHE:
        return _CACHE["m1t"], _CACHE["m2p"]
    n = np.arange(NG)
    k = np.arange(MD)
    C = np.cos(np.pi * np.outer(n, k) / (NG - 1))
    c = np.full(NG, 2.0); c[0] = 1.0; c[-1] = 1.0
    c2 = np.full(MD, 2.0); c2[0] = 1.0
    M1T = (C * c[:, None]).astype(np.float32)              # [n, k]
    M2 = (C.T * c2[:, None]).astype(np.float32)            # [k, n]
    m1t = np.ascontiguousarray(M1T.reshape(16, 128, MD).transpose(1, 0, 2))
    m2p = np.ascontiguousarray(M2.reshape(4, 128, NG).transpose(1, 0, 2))
    _CACHE["m1t"], _CACHE["m2p"] = m1t, m2p
    return m1t, m2p


def _build_nc(reps: int = 1, phases=("t1s1", "s2", "t2", "s3")):
    nc = bass.Bass("TRN2", target_bir_lowering=False)
    x_s = nc.dram_tensor("x_s", [BPC * IC, NG], FP16, kind="ExternalInput")
    wt = nc.dram_tensor("wt", [P, 64 * 256], FP16, kind="ExternalInput")
    m1t = nc.dram_tensor("m1t", [P, 16 * MD], FP16, kind="ExternalInput")
    m2p = nc.dram_tensor("m2p", [P, 4 * NG], FP16, kind="ExternalInput")
    idm = nc.dram_tensor("idm", [P, P], FP16, kind="ExternalInput")
    o_s = nc.dram_tensor("o_s", [BPC * OC, NG], FP16, kind="ExternalOutput")

    aps = dict(
        x_ap=x_s.ap(),
        wt_ap=wt.ap(),
        m1t_ap=m1t.ap().rearrange("p (j k) -> p j k", j=16),
        m2p_ap=m2p.ap().rearrange("p (c n) -> p c n", c=4),
        o_ap=o_s.ap(),
    )

    with SplitDrainTC(nc) as tc:
        with tc.tile_pool(name="const", bufs=1) as const:
            ident = const.tile([P, P], FP16)
            nc.sync.dma_start(ident[:], idm.ap())
            if reps == 1:
                _body(nc, tc, aps, ident, phases)
            else:
                with tc.For_i(0, reps, 1):
                    _body(nc, tc, aps, ident, phases)
    return nc


def _body(nc, tc, aps, ident, phases=("t1s1", "s2", "t2", "s3")):
    x_ap, wt_ap = aps["x_ap"], aps["wt_ap"]
    m1t_ap, m2p_ap, o_ap = aps["m1t_ap"], aps["m2p_ap"], aps["o_ap"]
    NCH = BPC * IC // P        # 128-row chunks of x (2 for BPC=4)
    NBP = BPC // 2             # batch pairs (2)

    with (
        tc.tile_pool(name="big", bufs=1) as big,
        tc.tile_pool(name="xb", bufs=1) as xb_pool,
        tc.tile_pool(name="m1", bufs=4) as m1_pool,
        tc.tile_pool(name="xt", bufs=6) as xt_pool,
        tc.tile_pool(name="m2", bufs=1) as m2_pool,
        tc.tile_pool(name="osb", bufs=4) as osb_pool,
    ):
        # xc pairs for block-diag S2: [128=(k2,i), b, kc]; k = k2*256 + kc
        xc2 = big.tile([P, BPC, 256], FP16)
        # block-diag weights [128=(k2,i), 128=(k2',o), kc] fp16 (zeros off-diag)
        wbd = big.tile([P, P, 256], FP16)
        # om, transposed om
        om2 = big.tile([P, BPC * 256], FP16)        # [(k2,o), kc*BPC+b]
        omts = [big.tile([P, 4, 2, 64], FP16, name=f"omt{bp}")
                for bp in range(NBP)]

        # -------- hoisted loads --------
        xbs = []
        xb = xb_pool.tile([P, NG], FP16, tag="xb0", name="xb0")
        nc.sync.dma_start(xb[:], x_ap[0:P, :])
        xbs.append(xb)
        m1js = {}
        for j in range(3):
            m1j = m1_pool.tile([P, MD], FP16, tag="m1", name=f"m1j{j}")
            nc.sync.dma_start(m1j[:], m1t_ap[:, j, :])
            m1js[j] = m1j
        for ch in range(1, NCH):
            xb = xb_pool.tile([P, NG], FP16, tag=f"xb{ch}", name=f"xb{ch}")
            nc.sync.dma_start(xb[:], x_ap[ch * P:(ch + 1) * P, :])
            xbs.append(xb)
        # diag blocks from compact host tensor; off-diag zero-filled on chip
        nc.vector.memset(wbd[0:64, 64:P, :], 0.0)
        nc.vector.memset(wbd[64:P, 0:64, :], 0.0)
        nc.scalar.dma_start(wbd[0:64, 0:64, :],
                            wt_ap[0:64, :].rearrange("p (o k) -> p o k", o=64))
        nc.scalar.dma_start(wbd[64:P, 64:P, :],
                            wt_ap[64:P, :].rearrange("p (o k) -> p o k", o=64))
        m2t = []
        for chv in range(4):
            t = m2_pool.tile([P, NG], FP16, tag=f"m2_{chv}", name=f"m2t{chv}")
            nc.scalar.dma_start(t[:], m2p_ap[:, chv, :])
            m2t.append(t)

        # ---------------- T1 + S1 ----------------
        if "t1s1" not in phases:
            return
        with (
            tc.tile_pool(name="ps_s1", bufs=1, space="PSUM") as ps_s1,
            tc.tile_pool(name="ps_xt", bufs=4, space="PSUM") as ps_xt,
        ):
            s1ps = [ps_s1.tile([P, MD], F32, tag=f"s1_{ch}", name=f"s1ps{ch}")
                    for ch in range(NCH)]
            for j in range(16):
                if j in m1js:
                    m1j = m1js[j]
                else:
                    m1j = m1_pool.tile([P, MD], FP16, tag="m1")
                    nc.sync.dma_start(m1j[:], m1t_ap[:, j, :])
                for ch in range(NCH):
                    tps = ps_xt.tile([P, P], FP16, tag="xtps")
                    nc.tensor.transpose(tps[:], xbs[ch][:, j * P:(j + 1) * P],
                                        ident[:])
                    xt = xt_pool.tile([P, P], FP16, tag="xt")
                    nc.vector.tensor_copy(out=xt[:], in_=tps[:])
                    nc.tensor.matmul(s1ps[ch][:], xt[:], m1j[:],
                                     start=(j == 0), stop=(j == 15))
            # evacuate (partition-shifted, cast) -> XC2 [(k2,i), b, kc]
            for ch in range(NCH):
                for b2 in range(2):
                    b = 2 * ch + b2
                    src = s1ps[ch][64 * b2:64 * b2 + 64, :]
                    nc.vector.tensor_copy(out=xc2[0:64, b, :], in_=src[:, 0:256])
                    nc.vector.tensor_copy(out=xc2[64:P, b, :], in_=src[:, 256:MD])

        with (
            tc.tile_pool(name="ps_s2", bufs=2, space="PSUM") as ps_s2,
            tc.tile_pool(name="ps_t2", bufs=4, space="PSUM") as ps_t2,
            tc.tile_pool(name="ps_s3", bufs=2, space="PSUM") as ps_s3,
        ):
            # ---------------- S2 (block-diag fp16, 2 modes/matmul) ----------
            if "s2" not in phases:
                return
            for kq in range(4):
                p2 = ps_s2.tile([P, BPC * 64], F32, tag="s2")
                for kl in range(64):
                    kc = kq * 64 + kl
                    nc.tensor.matmul(
                        p2[:, kl * BPC:(kl + 1) * BPC],
                        wbd[:, :, kc],
                        xc2[:, :, kc],
                        start=True, stop=True)
                nc.any.tensor_copy(
                    out=om2[:, kq * 64 * BPC:(kq + 1) * 64 * BPC], in_=p2[:])

            # ---------------- T2 ----------------
            # om2[(k2,o), kc*BPC+b]; k = k2*256 + kcH*128 + kl; ch = k2*2 + kcH
            if "t2" not in phases:
                return
            for bp in range(NBP):
                for bo in range(2):
                    b = 2 * bp + bo
                    for k2 in range(2):
                        for kcH in range(2):
                            tps = ps_t2.tile([P, 64], FP16, tag="t2")
                            nc.tensor.transpose(
                                tps[:],
                                om2[64 * k2:64 * k2 + 64,
                                    kcH * 128 * BPC + b:
                                    (kcH + 1) * 128 * BPC:BPC],
                                ident[64 * k2:64 * k2 + 64,
                                      64 * k2:64 * k2 + 64])
                            nc.any.tensor_copy(
                                out=omts[bp][:, 2 * k2 + kcH, bo, :], in_=tps[:])

            # ---------------- S3 ----------------
            if "s3" not in phases:
                return
            for bp in range(NBP):
                for nb in range(4):
                    ps3 = ps_s3.tile([P, 512], F32, tag="s3")
                    for ch in range(4):
                        nc.tensor.matmul(
                            ps3[:],
                            omts[bp][:, ch, :, :],
                            m2t[ch][:, nb * 512:(nb + 1) * 512],
                            start=(ch == 0), stop=(ch == 3))
                    osb = osb_pool.tile([P, 512], FP16, tag="osb")
                    nc.any.tensor_copy(out=osb[:], in_=ps3[:])
                    nc.sync.dma_start(
                        o_ap[bp * P:(bp + 1) * P, nb * 512:(nb + 1) * 512],
                        osb[:])


def _wbd_host(w):
    """Compact block-diag fp16 weights: wbd[(k2,i), o*256+kc]."""
    wbd = np.zeros((P, 64, 256), np.float16)
    wr = w.reshape(IC, OC, 2, 256)  # [i, o, k2, kc]
    wbd[0:64] = wr[:, :, 0, :].astype(np.float16)
    wbd[64:P] = wr[:, :, 1, :].astype(np.float16)
    return np.ascontiguousarray(wbd).reshape(P, 64 * 256)


def _host_inputs(x, w):
    m1t, m2p = _constants()
    return {
        "m1t": m1t.reshape(P, 16 * MD).astype(np.float16),
        "m2p": m2p.reshape(P, 4 * NG).astype(np.float16),
        "idm": np.eye(P, dtype=np.float16),
        "wt": _wbd_host(w),
    }


def _get_exec():
    """Build the Bass module and its jitted 8-core PJRT executable once."""
    if "exec" in _CACHE:
        return _CACHE["exec"]
    install_neuronx_cc_hook()
    nc = _build_nc()
    partition_name = (nc.partition_id_tensor.name
                      if nc.partition_id_tensor else None)
    in_names, out_names, out_avals = [], [], []
    in_shapes = {}
    for alloc in nc.m.functions[0].allocations:
        if not isinstance(alloc, mybir.MemoryLocationSet):
            continue
        name = alloc.memorylocations[0].name
        if alloc.tensor_shape:
            in_shapes[name] = (tuple(alloc.tensor_shape),
                               mybir.dt.np(alloc.dtype))
        if alloc.kind == "ExternalInput":
            if name != partition_name:
                in_names.append(name)
        elif alloc.kind == "ExternalOutput":
            out_names.append(name)
            out_avals.append(jax.core.ShapedArray(
                tuple(alloc.tensor_shape), mybir.dt.np(alloc.dtype)))
    n_params = len(in_names)
    all_in_names = in_names + out_names
    if partition_name is not None:
        all_in_names.append(partition_name)

    def _bass_call(*args):
        operands = list(args)
        if partition_name is not None:
            operands.append(partition_id_tensor())
        outs = _bass_exec_p.bind(
            *operands,
            out_avals=tuple(out_avals),
            in_names=tuple(all_in_names),
            out_names=tuple(out_names),
            lowering_input_output_aliases=(),
            sim_require_finite=True,
            sim_require_nnan=True,
            nc=nc,
        )
        return tuple(outs)

    devices = jax.devices()[:NCORES]
    assert len(devices) == NCORES
    mesh = Mesh(np.asarray(devices), ("core",))
    spec = PartitionSpec("core")
    n_outs = len(out_names)
    sharding = NamedSharding(mesh, spec)

    def _fresh_jit():
        return jax.jit(
            shard_map(_bass_call, mesh=mesh,
                      in_specs=(spec,) * (n_params + n_outs),
                      out_specs=(spec,) * n_outs,
                      check_rep=False),
            keep_unused=True,
        )

    example = [
        jax.ShapeDtypeStruct(
            (NCORES * in_shapes[n][0][0],) + in_shapes[n][0][1:],
            in_shapes[n][1], sharding=sharding)
        for n in in_names + out_names
    ]
    try:
        sharded = fast_dispatch_compile(
            lambda: _fresh_jit().lower(*example).compile())
    except Exception:
        sharded = _fresh_jit()

    _CACHE["exec"] = (sharded, in_names, out_names, out_avals, sharding)
    return _CACHE["exec"]


def _dev_consts(sharding):
    """Input-independent device buffers: DCT matrices, identity, zero out."""
    if "dev_consts" in _CACHE:
        return _CACHE["dev_consts"]
    m1t, m2p = _constants()
    rep = lambda a: np.broadcast_to(a, (NCORES,) + a.shape).reshape(
        NCORES * a.shape[0], *a.shape[1:])
    consts = {
        "m1t": jax.device_put(
            rep(m1t.reshape(P, 16 * MD).astype(np.float16)), sharding),
        "m2p": jax.device_put(
            rep(m2p.reshape(P, 4 * NG).astype(np.float16)), sharding),
        "idm": jax.device_put(rep(np.eye(P, dtype=np.float16)), sharding),
        "o_s": jax.device_put(
            np.zeros((NCORES * BPC * OC, NG), np.float16), sharding),
    }
    jax.block_until_ready(list(consts.values()))
    _CACHE["dev_consts"] = consts
    return consts


def _dev_weights(w, sharding):
    """Weights-derived device buffer, cached while `weights` is unchanged."""
    cached = _CACHE.get("w_fp")
    if cached is not None and cached.shape == w.shape and np.array_equal(cached, w):
        return _CACHE["dev_wt"]
    wbd = _wbd_host(w)
    rep = np.broadcast_to(wbd, (NCORES,) + wbd.shape).reshape(
        NCORES * wbd.shape[0], *wbd.shape[1:])
    dev = jax.device_put(np.ascontiguousarray(rep), sharding)
    jax.block_until_ready(dev)
    _CACHE["w_fp"] = w.copy()
    _CACHE["dev_wt"] = dev
    return dev


def kernel(x: np.ndarray, weights: np.ndarray) -> np.ndarray:
    x = np.asarray(x, dtype=np.float32)
    w = np.ascontiguousarray(np.asarray(weights, dtype=np.float32))

    sharded, in_names, out_names, out_avals, sharding = _get_exec()
    consts = _dev_consts(sharding)
    wt_dev = _dev_weights(w, sharding)
    oi = out_names.index("o_s")

    # pipelined waves: upload wave k+1 while wave k executes/downloads
    # (the tunnel is full-duplex, so h2d and d2h stream concurrently)
    in_flight = []
    for ck in range(CHUNKS):
        xh = x[ck * NBC:(ck + 1) * NBC].reshape(
            NCORES * BPC * IC, NG).astype(np.float16)
        xd = jax.device_put(xh, sharding)
        args = {
            "x_s": xd,
            "wt": wt_dev,
            "m1t": consts["m1t"],
            "m2p": consts["m2p"],
            "idm": consts["idm"],
            "o_s": consts["o_s"],
        }
        outs = sharded(*[args[n] for n in in_names],
                       *[args[n] for n in out_names])
        outs[oi].copy_to_host_async()
        in_flight.append((xd, outs))

    out = np.empty((B, OC, NG), np.float32)
    for ck, (xd, outs) in enumerate(in_flight):
        o16 = np.asarray(outs[oi])
        out[ck * NBC:(ck + 1) * NBC] = o16.reshape(NBC, OC, NG)
        xd.delete()
    return out
